# revision 1
# baseline (speedup 1.0000x reference)
"""Trainium2 Bass kernel for a transformer decoder layer (self-attn + cross-attn + FFN).

Sharding: 8 cores; cores 0-3 handle batch 0, cores 4-7 batch 1; each core owns a
contiguous 512-token slice of queries/tokens for every row-wise op.  K/V are
computed per-batch on every core (replicated) from the raw dec/enc inputs, so no
device collectives are needed.

Layout: everything on-chip is feature-major (x^T: [d on partitions, t on free]).
Host pre-transposes inputs and weights; the output shard is written feature-major
and transposed back on the host.  LayerNorm statistics are computed with
ones-column matmuls (cross-partition sums); softmax normalization sums ride along
the AV matmul as an appended ones-column of V; all linear-layer biases are folded
into the matmul accumulation group as a rank-1 (bias-row x ones-row) update.
"""

import sys
import time

for _p in ("/opt/trn_rl_repo", "/root/.axon_site/_ro/trn_rl_repo"):
    if _p not in sys.path:
        sys.path.append(_p)

import numpy as np
import ml_dtypes

B, T, D, H, DH, FFN = 2, 2048, 1024, 16, 64, 4096
N_CORES = 8
CPB = N_CORES // B          # cores per batch
TL = T // CPB               # local tokens per core
DC = D // 128               # d-chunks (8)
KC = T // 128               # key chunks (16)
OC1 = FFN // 128            # fc1 out chunks (32)
NT = T // 512               # 512-wide column tiles over T
EPS = 1e-12
SCALE = 1.0 / 8.0           # 1/sqrt(DH)
MASK_NEG = -80000.0         # additive mask value (pre-scale); exp(-1e4) == 0

bf16 = ml_dtypes.bfloat16

_CACHE = {}


KSLICE = 2 * 128            # per-core K/V head-dim slice (2 pairs = 4 heads... 256 o-dims)
KPART = KSLICE * T          # gathered K region elems per rank
VPART = T * KSLICE          # gathered V region elems per rank
AGLEN = KPART + VPART


def _emit(ctx, tc, nc, aps, use_mask):
    import concourse.bass as bass
    from concourse import mybir
    from contextlib import ExitStack
    dt = mybir.dt
    AF = mybir.ActivationFunctionType
    Alu = mybir.AluOpType
    fp32 = dt.float32
    bf = dt.bfloat16

    consts = ctx.enter_context(tc.tile_pool(name="consts", bufs=1))
    sbA = ctx.enter_context(tc.tile_pool(name="sbA", bufs=1))   # long-lived acts
    smallp = ctx.enter_context(tc.tile_pool(name="smallp", bufs=2))
    drp = ctx.enter_context(tc.tile_pool(name="drp", bufs=2, space="DRAM"))
    drbig = ctx.enter_context(tc.tile_pool(name="drbig", bufs=1, space="DRAM"))
    ps_lin = ctx.enter_context(tc.tile_pool(name="ps_lin", bufs=2, space="PSUM"))
    ps_sc = ctx.enter_context(tc.tile_pool(name="ps_sc", bufs=2, space="PSUM"))
    ps_av = ctx.enter_context(tc.tile_pool(name="ps_av", bufs=2, space="PSUM"))

    # ---- constants ----
    ones_bf = consts.tile([1, 512], bf)
    nc.gpsimd.memset(ones_bf[:], 1.0)
    ones_f32c = consts.tile([128, 1], fp32)
    nc.gpsimd.memset(ones_f32c[:], 1.0)
    ones_bfc = consts.tile([128, 1], bf)
    nc.gpsimd.memset(ones_bfc[:], 1.0)
    eps_t = consts.tile([1, 1], fp32)
    nc.gpsimd.memset(eps_t[:], EPS)

    # packed bias rows: [saq(D) sao(D) caq(D) cao(D) b2(D) sak(256) sav(256)
    #                    cak(256) cav(256)]
    batile = consts.tile([1, 5 * D + 4 * KSLICE], bf, tag="batile")
    nc.sync.dma_start(batile[:], aps["biases_att"][:])
    _off = {}
    _o = 0
    for nm, ln in (("b_saq", D), ("b_sao", D), ("b_caq", D), ("b_cao", D),
                   ("b2r", D), ("b_sak", KSLICE), ("b_sav", KSLICE),
                   ("b_cak", KSLICE), ("b_cav", KSLICE)):
        _off[nm] = (_o, ln)
        _o += ln
    bias_rows = {nm: batile[:, o:o + ln] for nm, (o, ln) in _off.items()}
    gbe_t = [None]

    def gbe(i):
        if gbe_t[0] is None:
            t = consts.tile([128, 48], fp32, tag="gbe")
            nc.sync.dma_start(t[:], aps["gbe_all"][:])
            gbe_t[0] = t
        return gbe_t[0][:, i * DC:(i + 1) * DC]

    # ---- activations / persistent ----
    xloc_f = sbA.tile([128, DC, TL], fp32, tag="x1f")
    nc.sync.dma_start(xloc_f[:], aps["xlocT_f32"].rearrange("(c p) t -> p c t", p=128))
    xloc_b = sbA.tile([128, DC, TL], bf, tag="x1b")
    nc.sync.dma_start(xloc_b[:], aps["xlocT_bf"].rearrange("(c p) t -> p c t", p=128))

    def linear_fm(w_sb, x_sb, b_row, out_cb, n, drain):
        for oc in range(DC):
            p = ps_lin.tile([128, 512], fp32, tag="lin")
            for dc in range(DC):
                nc.tensor.matmul(
                    p[:, 0:n], w_sb[:, dc, oc * 128:(oc + 1) * 128],
                    x_sb[:, dc, 0:n], start=(dc == 0), stop=False)
            nc.tensor.matmul(
                p[:, 0:n], b_row[:, oc * 128:(oc + 1) * 128], ones_bf[:, 0:n],
                start=False, stop=True)
            drain(p, out_cb(oc))

    def act_drain(p, dst):
        nc.scalar.activation(out=dst, in_=p[:, 0:dst.shape[-1]], func=AF.Copy,
                             scale=1.0)

    # =========================== pipeline ===========================
    mask_sa = mask_ca = None
    if use_mask:
        mask_sa = sbA.tile([128, KC, TL], bf, tag="mask_sa")
        nc.sync.dma_start(mask_sa[:],
                          aps["amask_saT"].rearrange("(c p) t -> p c t", p=128))
        mask_ca = sbA.tile([128, KC, TL], bf, tag="mask_ca")
        nc.sync.dma_start(mask_ca[:],
                          aps["amask_caT"].rearrange("(c p) t -> p c t", p=128))

    with ExitStack() as attn_ctx:
        wpool = attn_ctx.enter_context(tc.tile_pool(name="wpool", bufs=2))
        expp = attn_ctx.enter_context(tc.tile_pool(name="expp", bufs=8))
        vahp = attn_ctx.enter_context(tc.tile_pool(name="vahp", bufs=2))
        ktp = attn_ctx.enter_context(tc.tile_pool(name="ktp", bufs=2))
        stgp = attn_ctx.enter_context(tc.tile_pool(name="stgp", bufs=2))
        agp = attn_ctx.enter_context(tc.tile_pool(name="agp", bufs=1, space="DRAM"))

        def weight_tile(name):
            w = wpool.tile([128, DC, D], bf, tag="w")
            nc.sync.dma_start(w[:], aps[name].rearrange("(c p) o -> p c o", p=128))
            return w

        def kv_part(src_sb, wk_name, bk, wv_name, bv):
            """Project this core's K/V head-slice and stage into an AG input.

            ag_in layout: [2, 128, T] K-part (o-chunk, p, t) followed by
            [KC, 128, KSLICE] V-part (kc, p, o)."""
            wk = wpool.tile([128, DC, KSLICE], bf, tag="wkv")
            nc.sync.dma_start(wk[:],
                              aps[wk_name].rearrange("(c p) o -> p c o", p=128))
            ag_in = agp.tile([AGLEN], bf, tag="ag_in")
            k_reg = ag_in[0:KPART].rearrange("(oc p t) -> oc p t", oc=2, p=128)
            v_reg = ag_in[KPART:AGLEN].rearrange("(kc p o) -> kc p o", kc=KC, p=128)
            for oc in range(2):
                for nt in range(NT):
                    p = ps_lin.tile([128, 512], fp32, tag="lin")
                    for dc in range(DC):
                        nc.tensor.matmul(
                            p[:], wk[:, dc, oc * 128:(oc + 1) * 128],
                            src_sb[:, dc, nt * 512:(nt + 1) * 512],
                            start=(dc == 0), stop=False)
                    nc.tensor.matmul(p[:], bias_rows[bk][:, oc * 128:(oc + 1) * 128],
                                     ones_bf[:], start=False, stop=True)
                    stg = stgp.tile([128, 512], bf, tag="stg")
                    nc.scalar.activation(out=stg[:], in_=p[:], func=AF.Copy,
                                         scale=1.0)
                    nc.sync.dma_start(k_reg[oc, :, nt * 512:(nt + 1) * 512], stg[:])
            wv = wpool.tile([128, DC, KSLICE], bf, tag="wkv")
            nc.sync.dma_start(wv[:],
                              aps[wv_name].rearrange("(c p) o -> p c o", p=128))
            for kc in range(KC):
                p = ps_lin.tile([128, 512], fp32, tag="lin")
                for dc in range(DC):
                    nc.tensor.matmul(
                        p[:, 0:KSLICE], src_sb[:, dc, kc * 128:(kc + 1) * 128],
                        wv[:, dc, :], start=(dc == 0), stop=False)
                nc.tensor.matmul(p[:, 0:KSLICE], ones_bf[:, 0:128],
                                 bias_rows[bv][:], start=False, stop=True)
                stg = stgp.tile([128, 512], bf, tag="stg")
                nc.scalar.activation(out=stg[:, 0:KSLICE], in_=p[:, 0:KSLICE],
                                     func=AF.Copy, scale=1.0)
                nc.sync.dma_start(v_reg[kc], stg[:, 0:KSLICE])
            ag_out = agp.tile([CPB, AGLEN], bf, tag="ag_out")
            nc.gpsimd.collective_compute(
                "AllGather", Alu.bypass,
                ins=[ag_in.opt()], outs=[ag_out.opt()],
                replica_groups=[list(range(CPB)),
                                list(range(CPB, 2 * CPB))])
            return ag_out

        def attention(qt, ag_out, mask_sb, out_cb):
            for pr in range(DC):
                ktp_t = ktp.tile([128, T], bf, tag="kth")
                k_src = ag_out[pr // 2, 0:KPART].rearrange(
                    "(oc p t) -> oc p t", oc=2, p=128)[pr % 2]
                nc.sync.dma_start(ktp_t[:], k_src)
                # QK^T for both heads of the pair, interleaved: the two heads'
                # matmuls carry 64-row tile positions (0,0)/(64,0) so the PE
                # runs them concurrently on independent sub-arrays.
                exp_ts = {0: [], 1: []}
                for kcg in range(4):
                    e2 = {}
                    for hf in range(2):
                        e_t = expp.tile([128, 4, TL], bf, tag="exp")
                        e2[hf] = e_t
                    exp_ts[0].append(e2[0])
                    exp_ts[1].append(e2[1])
                    for half in range(2):
                        ps2 = {}
                        for hf in range(2):
                            p_t = ps_sc.tile([128, 2, TL], fp32, tag="sc")
                            ps2[hf] = p_t
                        for j in range(2):
                            kc = 4 * kcg + 2 * half + j
                            for hf in range(2):
                                po = 64 * hf
                                nc.tensor.matmul(
                                    ps2[hf][:, j, :],
                                    ktp_t[po:po + 64, kc * 128:(kc + 1) * 128],
                                    qt[po:po + 64, pr, 0:TL],
                                    start=True, stop=True)
                                if mask_sb is not None:
                                    nc.vector.tensor_add(
                                        out=ps2[hf][:, j, :], in0=ps2[hf][:, j, :],
                                        in1=mask_sb[:, kc, :])
                        for hf in range(2):
                            nc.scalar.activation(
                                out=e2[hf][:, 2 * half:2 * half + 2, :].rearrange(
                                    "p a b -> p (a b)"),
                                in_=ps2[hf].rearrange("p a b -> p (a b)"),
                                func=AF.Exp, scale=SCALE)
                for hf in range(2):
                    h = 2 * pr + hf
                    po = 64 * hf
                    va_h = vahp.tile([128, KC, DH + 1], bf, tag="vah")
                    nc.gpsimd.memset(va_h[:, :, DH:DH + 1], 1.0)
                    v_src = ag_out[h // 4, KPART:AGLEN].rearrange(
                        "(kc p hl d) -> p kc hl d", kc=KC, p=128, hl=4)
                    nc.sync.dma_start(va_h[:, :, 0:DH], v_src[:, :, h % 4, :])
                    pav = ps_av.tile([DH + 1, TL], fp32, tag="av")
                    for kc in range(KC):
                        nc.tensor.matmul(pav[:], va_h[:, kc, :],
                                         exp_ts[hf][kc // 4][:, kc % 4, :],
                                         start=(kc == 0), stop=(kc == KC - 1))
                    rrow = smallp.tile([1, TL], fp32, tag="row")
                    nc.vector.reciprocal(out=rrow[:], in_=pav[DH:DH + 1, :])
                    rdr = drp.tile([1, TL], fp32, tag="dr")
                    nc.sync.dma_start(rdr[:], rrow[:])
                    rb = smallp.tile([64, TL], fp32, tag="bc")
                    nc.sync.dma_start(rb[:], rdr.to_broadcast([64, TL]))
                    dst = out_cb(pr)
                    nc.vector.tensor_mul(out=dst[po:po + 64, :], in0=pav[0:DH, :],
                                         in1=rb[:])

        def layernorm(x_sb, g, be, out_f, out_b):
            with ExitStack() as ln_ctx:
                lnp = ln_ctx.enter_context(tc.tile_pool(name="lnp", bufs=1))
                pm = ps_av.tile([1, TL], fp32, tag="av")
                for dc in range(DC):
                    nc.tensor.matmul(pm[:], ones_f32c[:], x_sb[:, dc, :],
                                     start=(dc == 0), stop=(dc == DC - 1))
                mrow = smallp.tile([1, TL], fp32, tag="row")
                nc.scalar.activation(out=mrow[:], in_=pm[:], func=AF.Copy,
                                     scale=1.0 / D)
                mdr = drp.tile([1, TL], fp32, tag="dr")
                nc.sync.dma_start(mdr[:], mrow[:])
                mb = smallp.tile([128, TL], fp32, tag="bc")
                nc.sync.dma_start(mb[:], mdr.to_broadcast([128, TL]))
                xc = lnp.tile([128, DC, TL], fp32, tag="xc")
                sq = lnp.tile([128, DC, TL], bf, tag="sq")
                for dc in range(DC):
                    nc.vector.tensor_sub(out=xc[:, dc, :], in0=x_sb[:, dc, :],
                                         in1=mb[:])
                    nc.vector.tensor_mul(out=sq[:, dc, :], in0=xc[:, dc, :],
                                         in1=xc[:, dc, :])
                pv = ps_av.tile([1, TL], fp32, tag="av")
                for dc in range(DC):
                    nc.tensor.matmul(pv[:], ones_bfc[:], sq[:, dc, :],
                                     start=(dc == 0), stop=(dc == DC - 1))
                srow = smallp.tile([1, TL], fp32, tag="row")
                nc.scalar.activation(out=srow[:], in_=pv[:], func=AF.Sqrt,
                                     scale=1.0 / D, bias=eps_t[:])
                rrow2 = smallp.tile([1, TL], fp32, tag="row")
                nc.vector.reciprocal(out=rrow2[:], in_=srow[:])
                rdr2 = drp.tile([1, TL], fp32, tag="dr")
                nc.sync.dma_start(rdr2[:], rrow2[:])
                rb2 = smallp.tile([128, TL], fp32, tag="bc")
                nc.sync.dma_start(rb2[:], rdr2.to_broadcast([128, TL]))
                for dc in range(DC):
                    nc.vector.tensor_mul(out=xc[:, dc, :], in0=xc[:, dc, :],
                                         in1=rb2[:])
                    nc.vector.tensor_scalar(
                        out=out_f[:, dc, :], in0=xc[:, dc, :],
                        scalar1=g[:, dc:dc + 1], scalar2=be[:, dc:dc + 1],
                        op0=Alu.mult, op1=Alu.add)
                if out_b is not None:
                    nc.scalar.activation(
                        out=out_b.rearrange("p c t -> p (c t)"),
                        in_=out_f.rearrange("p c t -> p (c t)"), func=AF.Copy,
                        scale=1.0)

        # --- K/V for both attentions (sharded + gathered); AGs overlap with
        # the Q projection and SA attention ---
        with tc.tile_pool(name="srcp", bufs=1) as srcp:
            dec_sb = srcp.tile([128, DC, T], bf, tag="src")
            nc.sync.dma_start(dec_sb[:],
                              aps["decT_bf"].rearrange("(c p) t -> p c t", p=128))
            ag_sa = kv_part(dec_sb, "w_sak", "b_sak", "w_sav", "b_sav")
            enc_sb = srcp.tile([128, DC, T], bf, tag="src")
            nc.sync.dma_start(enc_sb[:],
                              aps["encT_bf"].rearrange("(c p) t -> p c t", p=128))
            ag_ca = kv_part(enc_sb, "w_cak", "b_cak", "w_cav", "b_cav")

        qt_sa = sbA.tile([128, DC, TL], bf, tag="qt")
        wq = weight_tile("w_saq")
        linear_fm(wq, xloc_b, bias_rows["b_saq"], lambda oc: qt_sa[:, oc, :], TL,
                  act_drain)
        attn1 = sbA.tile([128, DC, TL], bf, tag="attn")
        attention(qt_sa, ag_sa, mask_sa, lambda pr: attn1[:, pr, :])

        # --- SA O-proj + residual -> x ---
        x_sb = sbA.tile([128, DC, TL], fp32, tag="x")
        wo = weight_tile("w_sao")
        for oc in range(DC):
            p = ps_lin.tile([128, 512], fp32, tag="lin")
            for dc in range(DC):
                nc.tensor.matmul(p[:, 0:TL], wo[:, dc, oc * 128:(oc + 1) * 128],
                                 attn1[:, dc, :], start=(dc == 0), stop=False)
            nc.tensor.matmul(p[:, 0:TL],
                             bias_rows["b_sao"][:, oc * 128:(oc + 1) * 128],
                             ones_bf[:, 0:TL], start=False, stop=True)
            nc.vector.tensor_add(out=x_sb[:, oc, :], in0=p[:, 0:TL],
                                 in1=xloc_f[:, oc, :])

        x1_f = sbA.tile([128, DC, TL], fp32, tag="x1f")
        x1_b = sbA.tile([128, DC, TL], bf, tag="x1b")
        layernorm(x_sb, gbe(0), gbe(1), x1_f, x1_b)

        # --- CA ---
        qt_ca = sbA.tile([128, DC, TL], bf, tag="qt")
        wqc = weight_tile("w_caq")
        linear_fm(wqc, x1_b, bias_rows["b_caq"], lambda oc: qt_ca[:, oc, :], TL,
                  act_drain)
        attn2 = sbA.tile([128, DC, TL], bf, tag="attn")
        attention(qt_ca, ag_ca, mask_ca, lambda pr: attn2[:, pr, :])

        woc = weight_tile("w_cao")
        x2_sb = sbA.tile([128, DC, TL], fp32, tag="x")
        for oc in range(DC):
            p = ps_lin.tile([128, 512], fp32, tag="lin")
            for dc in range(DC):
                nc.tensor.matmul(p[:, 0:TL], woc[:, dc, oc * 128:(oc + 1) * 128],
                                 attn2[:, dc, :], start=(dc == 0), stop=False)
            nc.tensor.matmul(p[:, 0:TL],
                             bias_rows["b_cao"][:, oc * 128:(oc + 1) * 128],
                             ones_bf[:, 0:TL], start=False, stop=True)
            nc.vector.tensor_add(out=x2_sb[:, oc, :], in0=p[:, 0:TL],
                                 in1=x1_f[:, oc, :])

        x2_f = sbA.tile([128, DC, TL], fp32, tag="x1f")
        x2_b = sbA.tile([128, DC, TL], bf, tag="x1b")
        layernorm(x2_sb, gbe(2), gbe(3), x2_f, x2_b)

    # --- FFN ---
    with ExitStack() as ffn_ctx:
        ffnp = ffn_ctx.enter_context(tc.tile_pool(name="ffnp", bufs=1))
        w1p = ffn_ctx.enter_context(tc.tile_pool(name="w1p", bufs=4))
        w2p = ffn_ctx.enter_context(tc.tile_pool(name="w2p", bufs=2))
        b1r = ffnp.tile([1, FFN], bf, tag="b1r")
        nc.sync.dma_start(b1r[:], aps["b1r"][:])
        ht = ffnp.tile([128, OC1, TL], bf, tag="ht")
        for oc in range(OC1):
            w1 = w1p.tile([128, DC, 128], bf, tag="w1")
            nc.sync.dma_start(
                w1[:],
                aps["w1t"].rearrange("(c p) o -> p c o",
                                     p=128)[:, :, oc * 128:(oc + 1) * 128])
            p = ps_lin.tile([128, 512], fp32, tag="lin")
            for dc in range(DC):
                nc.tensor.matmul(p[:, 0:TL], w1[:, dc, :], x2_b[:, dc, :],
                                 start=(dc == 0), stop=False)
            nc.tensor.matmul(p[:, 0:TL], b1r[:, oc * 128:(oc + 1) * 128],
                             ones_bf[:, 0:TL], start=False, stop=True)
            nc.scalar.activation(out=ht[:, oc, :], in_=p[:, 0:TL], func=AF.Relu,
                                 scale=1.0)
        y_sb = sbA.tile([128, DC, TL], fp32, tag="x")
        for oc in range(DC):
            w2 = w2p.tile([128, OC1, 128], bf, tag="w2")
            nc.sync.dma_start(
                w2[:],
                aps["w2t"].rearrange("(c p) o -> p c o",
                                     p=128)[:, :, oc * 128:(oc + 1) * 128])
            p = ps_lin.tile([128, 512], fp32, tag="lin")
            for kc in range(OC1):
                nc.tensor.matmul(p[:, 0:TL], w2[:, kc, :], ht[:, kc, :],
                                 start=(kc == 0), stop=False)
            nc.tensor.matmul(p[:, 0:TL], bias_rows["b2r"][:, oc * 128:(oc + 1) * 128],
                             ones_bf[:, 0:TL], start=False, stop=True)
            nc.vector.tensor_add(out=y_sb[:, oc, :], in0=p[:, 0:TL],
                                 in1=x2_f[:, oc, :])

        out_f = sbA.tile([128, DC, TL], fp32, tag="x1f")
        layernorm(y_sb, gbe(4), gbe(5), out_f, None)
        nc.sync.dma_start(aps["outT"].rearrange("(c p) t -> p c t", p=128), out_f[:])


def _build(use_mask):
    import concourse.bass as bass
    import concourse.tile as tile
    from concourse import bacc, mybir
    dt = mybir.dt
    nc = bacc.Bacc("TRN2", target_bir_lowering=False, debug=False,
                   num_devices=N_CORES)
    aps = {}

    def inp(name, shape, dtype):
        aps[name] = nc.dram_tensor(name, shape, dtype, kind="ExternalInput").ap()

    inp("decT_bf", [D, T], dt.bfloat16)
    inp("encT_bf", [D, T], dt.bfloat16)
    inp("xlocT_f32", [D, TL], dt.float32)
    inp("xlocT_bf", [D, TL], dt.bfloat16)
    for nm in ("w_saq", "w_sao", "w_caq", "w_cao"):
        inp(nm, [D, D], dt.bfloat16)
    for nm in ("w_sak", "w_sav", "w_cak", "w_cav"):
        inp(nm, [D, KSLICE], dt.bfloat16)
    inp("w1t", [D, FFN], dt.bfloat16)
    inp("w2t", [FFN, D], dt.bfloat16)
    inp("biases_att", [1, 5 * D + 4 * KSLICE], dt.bfloat16)
    inp("b1r", [1, FFN], dt.bfloat16)
    inp("gbe_all", [128, 48], dt.float32)
    if use_mask:
        inp("amask_saT", [T, TL], dt.bfloat16)
        inp("amask_caT", [T, TL], dt.bfloat16)
    aps["outT"] = nc.dram_tensor("outT", [D, TL], dt.float32,
                                 kind="ExternalOutput").ap()

    from contextlib import ExitStack
    with tile.TileContext(nc) as tc:
        with ExitStack() as ctx:
            _emit(ctx, tc, nc, aps, use_mask)
    nc.compile()
    return nc


def _make_runner(nc):
    import jax
    from jax.sharding import Mesh, PartitionSpec
    from jax.experimental.shard_map import shard_map
    from concourse import bass2jax, mybir

    bass2jax.install_neuronx_cc_hook()
    part_name = nc.partition_id_tensor.name if nc.partition_id_tensor else None
    in_names, out_names, out_avals = [], [], []
    for alloc in nc.m.functions[0].allocations:
        if not isinstance(alloc, mybir.MemoryLocationSet):
            continue
        name = alloc.memorylocations[0].name
        if alloc.kind == "ExternalInput":
            if name != part_name:
                in_names.append(name)
        elif alloc.kind == "ExternalOutput":
            out_names.append(name)
            out_avals.append(jax.core.ShapedArray(tuple(alloc.tensor_shape),
                                                  mybir.dt.np(alloc.dtype)))
    n_params = len(in_names)
    all_names = list(in_names + out_names)
    if part_name is not None:
        all_names.append(part_name)
    all_names = tuple(all_names)

    def _body(*args):
        operands = list(args)
        if part_name is not None:
            operands.append(bass2jax.partition_id_tensor())
        return tuple(bass2jax._bass_exec_p.bind(
            *operands, out_avals=tuple(out_avals), in_names=all_names,
            out_names=tuple(out_names), lowering_input_output_aliases=(),
            sim_require_finite=True, sim_require_nnan=True, nc=nc))

    devices = jax.devices()[:N_CORES]
    mesh = Mesh(np.asarray(devices), ("core",))
    spec = (PartitionSpec("core"),)
    nin = n_params + len(out_names)
    jfn = jax.jit(
        shard_map(_body, mesh=mesh, in_specs=spec * nin,
                  out_specs=spec * len(out_names), check_rep=False),
        donate_argnums=tuple(range(n_params, nin)), keep_unused=True)

    from jax.sharding import NamedSharding
    shard = NamedSharding(mesh, PartitionSpec("core"))

    def run(in_maps, timeit=False):
        concat_in = [np.concatenate([np.asarray(in_maps[c][n]) for c in range(N_CORES)],
                                    axis=0) for n in in_names]
        zeros = [np.zeros((N_CORES * a.shape[0],) + tuple(a.shape[1:]), a.dtype)
                 for a in out_avals]
        dev_in = [jax.device_put(a, shard) for a in concat_in]
        out = jfn(*dev_in, *[jax.device_put(z, shard) for z in zeros])
        jax.block_until_ready(out)
        times = []
        if timeit:
            # batched back-to-back dispatch; the k-slope of total wall time
            # isolates per-exec device time from fixed batch overhead.
            for _rep in range(2):
                totals = {}
                for k in (4, 44):
                    zsets = [[jax.device_put(z, shard) for z in zeros]
                             for _ in range(k)]
                    jax.block_until_ready(zsets)
                    t0 = time.perf_counter()
                    outs = [jfn(*dev_in, *zs) for zs in zsets]
                    jax.block_until_ready(outs)
                    totals[k] = time.perf_counter() - t0
                times.append((totals[44] - totals[4]) / 40.0)
        per_core = [{n: np.asarray(out[i]).reshape(N_CORES, *out_avals[i].shape)[c]
                     for i, n in enumerate(out_names)} for c in range(N_CORES)]
        return per_core, times

    return run


def _prep_inputs(dec, enc, t_mask, s_mask, weights):
    """Returns (in_maps, use_mask)."""
    use_mask_sa = not bool(np.all(t_mask != 0))
    use_mask_ca = not bool(np.all(s_mask != 0))
    use_mask = use_mask_sa or use_mask_ca

    kvT = weights["_kvT"]
    bias = weights["_bias"]
    shared = {k: v for k, v in weights.items() if not k.startswith("_")}
    in_maps = []
    for c in range(N_CORES):
        b, s = c // CPB, c % CPB
        rows = slice(s * TL, (s + 1) * TL)
        osl = slice(s * KSLICE, (s + 1) * KSLICE)
        m = dict(shared)
        for nm in ("w_sak", "w_sav", "w_cak", "w_cav"):
            m[nm] = np.ascontiguousarray(kvT[nm][:, osl])
        batt = np.concatenate([
            bias["b_saq"], bias["b_sao"], bias["b_caq"], bias["b_cao"],
            bias["b2r"], bias["b_sak"][osl], bias["b_sav"][osl],
            bias["b_cak"][osl], bias["b_cav"][osl]])
        m["biases_att"] = batt.reshape(1, -1).astype(bf16)
        decT = np.ascontiguousarray(dec[b].T)
        m["decT_bf"] = decT.astype(bf16)
        m["encT_bf"] = np.ascontiguousarray(enc[b].T).astype(bf16)
        xloc = np.ascontiguousarray(dec[b, rows].T)
        m["xlocT_f32"] = xloc
        m["xlocT_bf"] = xloc.astype(bf16)
        if use_mask:
            am_sa = ((t_mask[0, 0] == 0) * MASK_NEG).astype(np.float32)
            am_ca = ((s_mask[0, 0] == 0) * MASK_NEG).astype(np.float32)
            m["amask_saT"] = np.ascontiguousarray(am_sa[rows].T).astype(bf16)
            m["amask_caT"] = np.ascontiguousarray(am_ca[rows].T).astype(bf16)
        in_maps.append(m)
    return in_maps, use_mask


def _pack_weights(kw):
    """Shared (core-independent) weight tensors."""
    w = {}
    for src, dst in (("sa_wq", "w_saq"), ("sa_wo", "w_sao"), ("ca_wq", "w_caq"),
                     ("ca_wo", "w_cao"), ("w1", "w1t"), ("w2", "w2t")):
        w[dst] = np.ascontiguousarray(np.asarray(kw[src]).T).astype(bf16)
    w["b1r"] = np.asarray(kw["b1"]).reshape(1, -1).astype(bf16)
    gbe = [np.asarray(kw[k]).astype(np.float32).reshape(DC, 128).T for k in
           ("g1", "be1", "g2", "be2", "g3", "be3")]
    w["gbe_all"] = np.ascontiguousarray(np.concatenate(gbe, axis=1))
    # K/V weights are head-sharded per core: slices added in _prep_inputs.
    w["_kvT"] = {nm: np.ascontiguousarray(np.asarray(kw[src]).T).astype(bf16)
                 for src, nm in (("sa_wk", "w_sak"), ("sa_wv", "w_sav"),
                                 ("ca_wk", "w_cak"), ("ca_wv", "w_cav"))}
    w["_bias"] = {nm: np.asarray(kw[src]).reshape(-1).astype(np.float32)
                  for src, nm in (("sa_bq", "b_saq"), ("sa_bo", "b_sao"),
                                  ("ca_bq", "b_caq"), ("ca_bo", "b_cao"),
                                  ("b2", "b2r"), ("sa_bk", "b_sak"),
                                  ("sa_bv", "b_sav"), ("ca_bk", "b_cak"),
                                  ("ca_bv", "b_cav"))}
    return w


def _get_runner(use_mask):
    key = bool(use_mask)
    if key not in _CACHE:
        nc = _build(key)
        _CACHE[key] = _make_runner(nc)
    return _CACHE[key]


def kernel(dec, enc, t_mask, s_mask, timeit=False, **kw):
    dec = np.asarray(dec, dtype=np.float32)
    enc = np.asarray(enc, dtype=np.float32)
    weights = _pack_weights(kw)
    in_maps, use_mask = _prep_inputs(dec, enc, np.asarray(t_mask),
                                     np.asarray(s_mask), weights)
    run = _get_runner(use_mask)
    per_core, times = run(in_maps, timeit=timeit)
    out = np.empty((B, T, D), np.float32)
    for c in range(N_CORES):
        b, s = c // CPB, c % CPB
        out[b, s * TL:(s + 1) * TL] = per_core[c]["outT"].T
    if timeit:
        kernel._last_times = times
    return out

